# revision 3
# baseline (speedup 1.0000x reference)
"""Trainium2 Bass kernel for the char-decoder model (8 NeuronCores, SPMD).

Device: conv front-end + 6-layer BERT decoder + nar projection, data-parallel
over batch (4/core), feature-major activations, bf16 GEMMs with fp32 PSUM.
Host: char-LSTM recurrence + output projection/NLL (numpy), pending the
tensor-parallel on-device LSTM.
"""

import os
import numpy as np
import ml_dtypes

import concourse.bass as bass
import concourse.bacc as bacc
import concourse.mybir as mybir
import concourse.tile as tile
from concourse.bass import ds, ts
from concourse.bass_utils import run_bass_kernel_spmd
from concourse.masks import make_identity

F32 = mybir.dt.float32
BF16 = mybir.dt.bfloat16
AF = mybir.ActivationFunctionType
ALU = mybir.AluOpType
BF = ml_dtypes.bfloat16

DIM, HEADS, HD, FF, E, V, SHRINK, L, HALF = 1024, 16, 64, 2048, 128, 256, 5, 6, 512
B, S, T, SRC, NCORE = 32, 1000, 200, 256, 8
BC = B // NCORE            # 4 batches/core
TOK = BC * T               # 800
ETOK = BC * SRC            # 1024
CTOK = BC * S              # 4000
SCALE = 1.0 / 8.0

_CACHE = {}


def _tokwins(n, w=512):
    o = 0
    while o < n:
        yield o, min(w, n - o)
        o += w


def build_program():
    nc = bacc.Bacc(None, target_bir_lowering=False, num_devices=NCORE,
                   enable_partition_id=True)

    def din(name, shape, dt=BF16):
        return nc.dram_tensor(name, shape, dt, kind="ExternalInput")

    embT = din("embT", [E, CTOK])
    convmask = din("convmask", [1, CTOK], F32)
    posT = din("posT", [128, 8, TOK])
    selfmaskT = din("selfmaskT", [128, 2, BC, T])
    crossmaskT = din("crossmaskT", [128, 2, BC], F32)
    encT = din("encT", [128, 8, ETOK])

    wc = din("wc", [E, DIM])
    bcT = din("bcT", [128, 8], F32)
    lng = din("lng", [128, 19, 8], F32)
    lnb = din("lnb", [128, 19, 8], F32)
    wqkv = din("wqkv", [L, DIM + 1, 3 * DIM])     # row DIM = bqkv (for V bias)
    bqkT = din("bqkT", [128, L, 16], F32)
    wo = din("wo", [L, DIM + 1, DIM])             # row DIM = bo
    co = din("co", [L, DIM + 1, DIM])             # row DIM = cbo
    cq = din("cq", [L, DIM, DIM])
    cbqT = din("cbqT", [128, L, 8], F32)
    ckv = din("ckv", [L, DIM + 1, 2 * DIM])       # row DIM = cbkv (for V bias)
    cbkvkT = din("cbkvkT", [128, L, 8], F32)
    w1 = din("w1", [L, DIM, FF])
    b1T = din("b1T", [128, L, 16], F32)
    w2 = din("w2", [L, FF + 1, DIM])              # row FF = b2
    wnar = din("wnar", [DIM, SHRINK * HALF])
    bnarT = din("bnarT", [128, 20], F32)

    lstm_embT = din("lstm_embT", [B, SHRINK, E, T])
    whhT = din("whhT", [DIM, 512])
    wihT = din("wihT", [E + HALF + 1, 512])
    onehotT = din("onehotT", [128, 2, CTOK])
    tmaskT = din("tmaskT", [1, CTOK], F32)
    wout = din("wout", [DIM + HALF + 1, V])

    out_nll = nc.dram_tensor("out_nll", [1, 1], F32, kind="ExternalOutput")
    cs_bounce = nc.dram_tensor("cs_bounce", [128, 20 * TOK], BF16)
    csT_all = nc.dram_tensor("csT_all", [128 * NCORE, 20 * TOK], BF16,
                             addr_space="Shared")
    xpart = nc.dram_tensor("xpart", [S, B, 512], F32)

    recv_sem = nc.alloc_semaphore("recv_sem")
    send_sem = nc.alloc_semaphore("send_sem")
    prep_sem = nc.alloc_semaphore("prep_sem")
    xp_sem = nc.alloc_semaphore("xp_sem")
    mm_sem = nc.alloc_semaphore("mm_sem")
    petr_sem = nc.alloc_semaphore("petr_sem")
    dve1_sem = nc.alloc_semaphore("dve1_sem")
    dvec_sem = nc.alloc_semaphore("dvec_sem")
    dveh_sem = nc.alloc_semaphore("dveh_sem")
    dvehs_sem = nc.alloc_semaphore("dvehs_sem")
    dveself_sem = nc.alloc_semaphore("dveself_sem")
    actg_sem = nc.alloc_semaphore("actg_sem")
    acttc_sem = nc.alloc_semaphore("acttc_sem")
    actcp_sem = nc.alloc_semaphore("actcp_sem")

    with tile.TileContext(nc) as tc:
      with (
        tc.tile_pool(name="const", bufs=1) as cp,
        tc.tile_pool(name="xs", bufs=1) as xs,
        tc.tile_pool(name="lnw", bufs=1) as lnw,
        tc.tile_pool(name="psA", bufs=1, space="PSUM") as psA,
      ):
        ones = cp.tile([1, 512], BF16)
        onesF = cp.tile([1, 128], F32)
        onesD = cp.tile([128, 1], BF16)
        epst = cp.tile([1, 1], F32)
        lngt = cp.tile([128, 19, 8], F32)
        lnbt = cp.tile([128, 19, 8], F32)
        nc.vector.memset(ones[:], 1.0)
        nc.vector.memset(onesF[:], 1.0)
        nc.vector.memset(onesD[:], 1.0 / DIM)
        nc.vector.memset(epst[:], 1e-12)
        nc.sync.dma_start(lngt[:], lng[:])
        nc.sync.dma_start(lnbt[:], lnb[:])

        x = xs.tile([128, 8, TOK], BF16)
        s_t = xs.tile([128, 8, TOK], BF16)

        def pst(tag):
            return psA.tile([128, 512], F32, tag=tag, name="ps_" + tag)

        def layernorm(xout, sin, ln_idx, sq_t):
            nc.scalar.activation(sq_t[:], sin[:], AF.Square)
            for w0 in (0, 400):
                wn = 400
                mps = pst("t4")[:1, :wn]
                qps = pst("t5")[:1, :wn]
                for kc in range(8):
                    nc.tensor.matmul(mps, onesD[:], sin[:, kc, w0:w0 + wn],
                                     start=(kc == 0), stop=(kc == 7))
                for kc in range(8):
                    nc.tensor.matmul(qps, onesD[:], sq_t[:, kc, w0:w0 + wn],
                                     start=(kc == 0), stop=(kc == 7))
                mean_s = lnw.tile([1, 400], F32, tag="ln_mean")
                m2 = lnw.tile([1, 400], F32, tag="ln_m2")
                istd = lnw.tile([1, 400], F32, tag="ln_istd")
                mi = lnw.tile([1, 400], F32, tag="ln_mi")
                nc.vector.tensor_copy(mean_s[:], mps)
                nc.vector.tensor_tensor(m2[:], mean_s[:], mean_s[:], ALU.mult)
                nc.vector.tensor_tensor(m2[:], qps, m2[:], ALU.subtract)
                nc.scalar.activation(m2[:], m2[:], AF.Sqrt, bias=epst[:])
                nc.vector.reciprocal(istd[:], m2[:])
                nc.vector.tensor_tensor(mi[:], mean_s[:], istd[:], ALU.mult)
                ibc = pst("t6")[:, :wn]
                mbc = pst("t7")[:, :wn]
                nc.tensor.matmul(ibc, onesF[:, 0:128], istd[:], start=True, stop=True)
                nc.tensor.matmul(mbc, onesF[:, 0:128], mi[:], start=True, stop=True)
                for kc in range(8):
                    t1 = lnw.tile([128, 400], BF16, tag="ln_t1")
                    nc.vector.tensor_tensor(t1[:], sin[:, kc, w0:w0 + wn], ibc, ALU.mult)
                    nc.vector.tensor_tensor(t1[:], t1[:], mbc, ALU.subtract)
                    nc.vector.tensor_scalar(
                        xout[:, kc, w0:w0 + wn], t1[:],
                        lngt[:, ln_idx, kc:kc + 1], lnbt[:, ln_idx, kc:kc + 1],
                        ALU.mult, ALU.add)

        # ================= phase 0: conv + pool + embed =================
        with (
            tc.tile_pool(name="p0", bufs=1) as p0,
            tc.tile_pool(name="p0w", bufs=1) as p0w,
        ):
            embt = p0.tile([E, CTOK], BF16)
            nc.sync.dma_start(embt[:], embT[:])
            wcs = p0w.tile([E, DIM], BF16, tag="wc")
            nc.sync.dma_start(wcs[:], wc[:])
            bct = p0w.tile([128, 8], F32, tag="bct")
            nc.sync.dma_start(bct[:], bcT[:])
            pos = p0.tile([128, 8, TOK], BF16)
            nc.sync.dma_start(pos[:], posT[:])
            cmsk = p0w.tile([1, CTOK], F32, tag="cmsk")
            nc.sync.dma_start(cmsk[:], convmask[:])
            mbc_t = p0.tile([128, CTOK], BF16)
            for wi, (w0, wn) in enumerate(_tokwins(CTOK, 500)):
                mb = pst("t2")[:, :wn]
                nc.tensor.matmul(mb, onesF[:, 0:128], cmsk[:, w0:w0 + wn],
                                 start=True, stop=True)
                nc.scalar.copy(mbc_t[:, w0:w0 + wn], mb)

            for mc in range(8):
                hm = p0.tile([128, CTOK], BF16, tag="hconv")
                for wi, (w0, wn) in enumerate(_tokwins(CTOK, 500)):
                    ps = pst("t0" if wi % 2 == 0 else "t1")[:, :wn]
                    nc.tensor.matmul(ps, wcs[:, mc * 128:(mc + 1) * 128],
                                     embt[:, w0:w0 + wn], start=True, stop=True)
                    nc.scalar.activation(hm[:, w0:w0 + wn], ps, AF.Relu,
                                         bias=bct[:, mc:mc + 1])
                nc.vector.tensor_tensor(hm[:], hm[:], mbc_t[:], ALU.mult)
                nc.vector.tensor_reduce(
                    s_t[:, mc, :], hm[:].rearrange("p (t s) -> p t s", s=SHRINK),
                    mybir.AxisListType.X, ALU.max)
            nc.vector.tensor_tensor(s_t[:], s_t[:], pos[:], ALU.add)
            sq0 = p0.tile([128, 8, TOK], BF16)
            layernorm(x, s_t, 0, sq0)

        # ================= phases 1+2: bert layers + nar =================
        with (
            tc.tile_pool(name="wsl", bufs=18) as wsl,
            tc.tile_pool(name="wbias", bufs=2) as wbias,
            tc.tile_pool(name="vsl", bufs=1) as vsl,
            tc.tile_pool(name="qkv", bufs=1) as qkvp,
            tc.tile_pool(name="att", bufs=3) as attp,
            tc.tile_pool(name="f1p", bufs=1) as f1p,
            tc.tile_pool(name="bias", bufs=1) as biasp,
        ):
            QT = qkvp.tile([128, 8, TOK], BF16)
            KT = qkvp.tile([128, 8, TOK], BF16)
            Vt = qkvp.tile([128, BC, 2, HEADS, 65], BF16)
            KcT = qkvp.tile([128, 8, ETOK], BF16)
            Vct = qkvp.tile([128, BC, 2, HEADS, 65], BF16)
            attnT = qkvp.tile([128, 8, TOK], BF16)
            f1T = f1p.tile([128, 16, TOK], BF16)
            enct = qkvp.tile([128, 8, ETOK], BF16)
            smt = qkvp.tile([128, 2, BC, T], BF16)
            cmt = qkvp.tile([128, 2, BC], F32)
            bqk_t = biasp.tile([128, L, 16], F32)
            cbq_t = biasp.tile([128, L, 8], F32)
            cbkvk_t = biasp.tile([128, L, 8], F32)
            b1_t = biasp.tile([128, L, 16], F32)
            bnar_t = biasp.tile([128, 20], F32)
            nc.sync.dma_start(smt[:], selfmaskT[:])
            nc.sync.dma_start(cmt[:], crossmaskT[:])
            nc.sync.dma_start(enct[:], encT[:])
            nc.sync.dma_start(bqk_t[:], bqkT[:])
            nc.sync.dma_start(cbq_t[:], cbqT[:])
            nc.sync.dma_start(cbkvk_t[:], cbkvkT[:])
            nc.sync.dma_start(b1_t[:], b1T[:])
            nc.sync.dma_start(bnar_t[:], bnarT[:])

            def gemm_b(wdram, kcs, rhs_fn, mcs, out_cb, bias_row=None,
                       ntok=TOK, ntw=512):
                """feature-major GEMM: psum[mc] = sum_kc W[kc,:,mc*128:]^T @ rhs(kc)"""
                M = wdram.shape[-1]
                brow = None
                if bias_row is not None:
                    brow = wbias.tile([1, M], BF16, tag="wbias", name="brow")
                    nc.sync.dma_start(brow[:], wdram[bias_row:bias_row + 1, :])
                for mc in range(mcs):
                    wts = []
                    for kc in range(kcs):
                        wt = wsl.tile([128, 128], BF16, tag="wt", name="wt")
                        nc.sync.dma_start(
                            wt[:], wdram[kc * 128:(kc + 1) * 128,
                                         mc * 128:(mc + 1) * 128])
                        wts.append(wt)
                    for wi, (w0, wn) in enumerate(_tokwins(ntok, ntw)):
                        p = pst("t0" if (mc + wi) % 2 == 0 else "t1")[:, :wn]
                        for kc in range(kcs):
                            nc.tensor.matmul(
                                p, wts[kc], rhs_fn(kc, w0, wn),
                                start=(kc == 0),
                                stop=(kc == kcs - 1 and brow is None))
                        if brow is not None:
                            nc.tensor.matmul(p, brow[:, mc * 128:(mc + 1) * 128],
                                             ones[0:1, :wn], start=False, stop=True)
                        out_cb(mc, w0, wn, p)

            def gemm_a_v(wdram_v, rhs_x, vdst, ntok_grp, bias_row):
                """token-major V gemm: for each batch b and kp-chunk, psum
                [kn, 512] = x_chunk^T @ Wv, written into vdst[., b, chunk, h, d]."""
                slabs = []
                for kc in range(8):
                    sl = vsl.tile([128, DIM], BF16, tag=f"wslV{kc}", name="sl")
                    nc.sync.dma_start(sl[:], wdram_v[kc * 128:(kc + 1) * 128, :])
                    slabs.append(sl)
                brow = vsl.tile([1, DIM], BF16, tag="wslVb", name="brow")
                nc.sync.dma_start(brow[:], wdram_v[bias_row:bias_row + 1, :])
                kplens = (128, ntok_grp - 128)
                for b in range(BC):
                    for kchunk in range(2):
                        kn = kplens[kchunk]
                        t0 = b * ntok_grp + kchunk * 128
                        for w in range(2):
                            p = pst("t0" if w == 0 else "t1")[:kn, :]
                            for kc in range(8):
                                nc.tensor.matmul(
                                    p, rhs_x(kc, t0, kn),
                                    slabs[kc][:, w * 512:(w + 1) * 512],
                                    start=(kc == 0), stop=False)
                            nc.tensor.matmul(p, ones[0:1, :kn],
                                             brow[:, w * 512:(w + 1) * 512],
                                             start=False, stop=True)
                            nc.scalar.copy(
                                vdst[:kn, b, kchunk, w * 8:(w + 1) * 8, 0:64],
                                p.rearrange("k (h d) -> k h d", d=64))
                # ones column for the fused sum-of-exp row
                nc.vector.memset(vdst[:, :, :, :, 64:65], 1.0)

            def attention(src_is_self):
                kt = KT if src_is_self else KcT
                vt = Vt if src_is_self else Vct
                nkp = T if src_is_self else SRC
                kplens = (128, nkp - 128)
                for b in range(BC):
                    for h in range(HEADS):
                        hp, hc = (h % 2) * 64, h // 2
                        et = attp.tile([128, 2, T], BF16, tag="et")
                        pso = pst("t2")[:65, :T]
                        for kchunk in range(2):
                            kn = kplens[kchunk]
                            base = b * nkp + kchunk * 128
                            pss = pst("t0" if kchunk == 0 else "t1")[:kn, :T]
                            nc.tensor.matmul(
                                pss, kt[hp:hp + 64, hc, base:base + kn],
                                QT[hp:hp + 64, hc, b * T:(b + 1) * T],
                                start=True, stop=True)
                            nc.scalar.activation(et[:kn, kchunk, :], pss, AF.Exp,
                                                 scale=SCALE)
                            if src_is_self:
                                nc.vector.tensor_tensor(
                                    et[:kn, kchunk, :], et[:kn, kchunk, :],
                                    smt[:kn, kchunk, b, :], ALU.mult)
                            else:
                                nc.vector.tensor_scalar_mul(
                                    et[:kn, kchunk, :], et[:kn, kchunk, :],
                                    cmt[:kn, kchunk, b:b + 1])
                            nc.tensor.matmul(
                                pso, vt[:kn, b, kchunk, h, :], et[:kn, kchunk, :],
                                start=(kchunk == 0), stop=(kchunk == 1))
                        rc = attp.tile([1, T], F32, tag="rc")
                        nc.vector.reciprocal(rc[:], pso[64:65, :])
                        prb = pst("t3")[:64, :T]
                        nc.tensor.matmul(prb, onesF[:, 0:64], rc[:],
                                         start=True, stop=True)
                        rb = attp.tile([64, T], F32, tag="rb")
                        nc.scalar.copy(rb[:], prb)
                        nc.vector.tensor_tensor(
                            attnT[hp:hp + 64, hc, b * T:(b + 1) * T],
                            pso[0:64, :], rb[:], ALU.mult)

            x_rhs = lambda kc, w0, wn: x[:, kc, w0:w0 + wn]

            for l in range(L):
                # ---- self attention ----
                def qk_out(mc, w0, wn, p, l=l):
                    dst = QT if mc < 8 else KT
                    nc.scalar.activation(dst[:, mc % 8, w0:w0 + wn], p, AF.Identity,
                                         bias=bqk_t[:, l, mc:mc + 1])
                gemm_b(wqkv[l, :, 0:2 * DIM], 8, x_rhs, 16, qk_out)
                gemm_a_v(wqkv[l, :, 2 * DIM:3 * DIM],
                         lambda kc, t0, kn: x[:, kc, t0:t0 + kn], Vt, T, DIM)
                attention(True)

                def o_out(mc, w0, wn, p):
                    nc.vector.tensor_tensor(s_t[:, mc, w0:w0 + wn], p,
                                            x[:, mc, w0:w0 + wn], ALU.add)
                gemm_b(wo[l], 8,
                       lambda kc, w0, wn: attnT[:, kc, w0:w0 + wn], 8, o_out,
                       bias_row=DIM)
                layernorm(x, s_t, 1 + 3 * l, attnT)

                # ---- cross attention ----
                def q_out(mc, w0, wn, p, l=l):
                    nc.scalar.activation(QT[:, mc, w0:w0 + wn], p, AF.Identity,
                                         bias=cbq_t[:, l, mc:mc + 1])
                gemm_b(cq[l], 8, x_rhs, 8, q_out)

                def kc_out(mc, w0, wn, p, l=l):
                    nc.scalar.activation(KcT[:, mc, w0:w0 + wn], p, AF.Identity,
                                         bias=cbkvk_t[:, l, mc:mc + 1])
                gemm_b(ckv[l][:, 0:DIM], 8,
                       lambda kc, w0, wn: enct[:, kc, w0:w0 + wn], 8, kc_out,
                       ntok=ETOK)
                gemm_a_v(ckv[l][:, DIM:2 * DIM],
                         lambda kc, t0, kn: enct[:, kc, t0:t0 + kn], Vct, SRC, DIM)
                attention(False)

                def co_out(mc, w0, wn, p):
                    nc.vector.tensor_tensor(s_t[:, mc, w0:w0 + wn], p,
                                            x[:, mc, w0:w0 + wn], ALU.add)
                gemm_b(co[l], 8,
                       lambda kc, w0, wn: attnT[:, kc, w0:w0 + wn], 8, co_out,
                       bias_row=DIM)
                layernorm(x, s_t, 2 + 3 * l, attnT)

                # ---- ffn ----
                def f1_out(mc, w0, wn, p, l=l):
                    nc.scalar.activation(f1T[:, mc, w0:w0 + wn], p, AF.Relu,
                                         bias=b1_t[:, l, mc:mc + 1])
                gemm_b(w1[l], 8, x_rhs, 16, f1_out)

                def f2_out(mc, w0, wn, p):
                    nc.vector.tensor_tensor(s_t[:, mc, w0:w0 + wn], p,
                                            x[:, mc, w0:w0 + wn], ALU.add)
                gemm_b(w2[l], 16,
                       lambda kc, w0, wn: f1T[:, kc, w0:w0 + wn], 8, f2_out,
                       bias_row=FF)
                layernorm(x, s_t, 3 + 3 * l, attnT)

            # ---- nar projection -> char states ----
            ocs = cs_bounce[:].rearrange("p (c t) -> p c t", t=TOK)

            def nar_out(mc, w0, wn, p):
                stg = f1p.tile([128, 512], BF16, tag="narstg")
                nc.scalar.activation(stg[:, :wn], p, AF.Identity,
                                     bias=bnar_t[:, mc:mc + 1])
                nc.sync.dma_start(ocs[:, mc, w0:w0 + wn], stg[:, :wn])
            gemm_b(wnar, 8, x_rhs, 20, nar_out)
            nc.gpsimd.collective_compute(
                "AllGather", ALU.bypass,
                replica_groups=[list(range(NCORE))],
                ins=[cs_bounce[:].opt()], outs=[csT_all[:].opt()])

        # ================= phase 3: x_part GEMM =================
        with (
            tc.tile_pool(name="p3", bufs=4) as p3,
            tc.tile_pool(name="p3w", bufs=1) as p3w,
        ):
            wih_t = []
            for kc in range(5):
                wt = p3w.tile([128, 512], BF16, tag=f"wih{kc}", name="wt")
                nc.sync.dma_start(wt[:], wihT[kc * 128:(kc + 1) * 128, :])
                wih_t.append(wt)
            wib = p3w.tile([1, 512], BF16, tag="wihb", name="wib")
            nc.sync.dma_start(wib[:], wihT[640:641, :])
            xpv = xpart[:].rearrange("(t s) b g -> s t b g", s=SHRINK)
            for bg in range(B):
                c_src, b_in = bg // BC, bg % BC
                for s_ in range(SHRINK):
                    for ci, (t0, tn) in enumerate(((0, 128), (128, 72))):
                        et = p3.tile([128, 128], BF16, tag="emb_l", name="et")
                        nc.sync.dma_start(et[:, :tn],
                                          lstm_embT[bg, s_, :, t0:t0 + tn])
                        lh = []
                        for dc in range(4):
                            ct = p3.tile([128, 128], BF16, tag=f"cs_l{dc}",
                                         name="ct")
                            off = (s_ * 4 + dc) * TOK + b_in * T + t0
                            nc.sync.dma_start(
                                ct[:, :tn],
                                csT_all[c_src * 128:(c_src + 1) * 128,
                                        off:off + tn])
                            lh.append(ct)
                        p = pst("t0" if (bg + s_ + ci) % 2 == 0 else "t1")
                        nc.tensor.matmul(p[:tn, :], et[:, :tn], wih_t[0][:],
                                         start=True, stop=False)
                        for dc in range(4):
                            nc.tensor.matmul(p[:tn, :], lh[dc][:, :tn],
                                             wih_t[1 + dc][:], start=False,
                                             stop=False)
                        nc.tensor.matmul(p[:tn, :], ones[0:1, :tn], wib[:],
                                         start=False, stop=True)
                        stg = p3.tile([128, 512], F32, tag="xstg", name="stg")
                        nc.scalar.copy(stg[:tn, :], p[:tn, :])
                        nc.sync.dma_start(xpv[s_, t0:t0 + tn, bg, :], stg[:tn, :])

        # ================= phase 4: LSTM (tensor-parallel) =================
        with (
            tc.tile_pool(name="lstm", bufs=1) as lp,
        ):
            hs_sbuf = lp.tile([128, 8, S, BC], BF16)
            whh_sb = lp.tile([128, 8, 512], BF16)
            nc.sync.dma_start(
                whh_sb[:], whhT[:].rearrange("(kc p) g -> p kc g", p=128))
            idf = lp.tile([128, 128], F32)
            make_identity(nc, idf[:])
            onesP = lp.tile([128, 1], BF16)
            nc.vector.memset(onesP[:], 1.0)
            hT_all = lp.tile([128, 2 * 8 * 32], BF16)
            c_st = lp.tile([32, 128], F32)
            gs = lp.tile([32, 512], F32)
            tc_t = lp.tile([32, 128], F32)
            h_sl = lp.tile([32, 128], F32)
            h_stage = lp.tile([128, 32], BF16)
            xp = lp.tile([32, 4, 512], F32)
            pgs = [pst("t0")[:32, :], pst("t1")[:32, :]]
            tps = [pst("t2")[:, :32], pst("t3")[:, :32]]
            hT_view = hT_all[:].rearrange("p (u kc b) -> p u kc b", u=2, b=32)

            with tc.tile_critical():
                nc.vector.memset(hT_all[:], 0.0)
                nc.vector.memset(c_st[:], 0.0)
                nc.all_engine_barrier()
                nc.all_core_barrier()
                pid_gp = nc.gpsimd.partition_id()
                pid_v = nc.vector.partition_id()
                for t in range(S):
                    buf, bufn = t % 2, (t + 1) % 2
                    pg = pgs[t % 2]
                    tp = tps[t % 2]
                    # xpart prefetch
                    if t >= 4:
                        nc.sync.wait_ge(dve1_sem, t - 3)
                    nc.sync.dma_start(xp[:, t % 4, :], xpart[t]).then_inc(
                        xp_sem, 16)
                    # recurrent matmuls
                    if t > 0:
                        nc.tensor.wait_ge(recv_sem, 14 * t)
                        nc.tensor.wait_ge(dveself_sem, t)
                    if t > 1:
                        nc.tensor.wait_ge(dve1_sem, t - 1)
                    for kc in range(8):
                        mm = nc.tensor.matmul(
                            pg[:], hT_all[:, ds(buf * 256 + kc * 32, 32)],
                            whh_sb[:, kc, :], start=(kc == 0), stop=(kc == 7))
                    mm.then_inc(mm_sem, 1)
                    # gates = psum + x_part
                    nc.vector.wait_ge(mm_sem, t + 1)
                    nc.vector.wait_ge(xp_sem, 16 * (t + 1))
                    if t > 0:
                        nc.vector.wait_ge(actg_sem, t)
                    nc.vector.tensor_tensor(gs[:], pg[:], xp[:, t % 4, :],
                                            ALU.add).then_inc(dve1_sem, 1)
                    # activations
                    nc.scalar.wait_ge(dve1_sem, t + 1)
                    nc.scalar.activation(gs[:, 0:384], gs[:, 0:384], AF.Sigmoid)
                    nc.scalar.activation(gs[:, 384:512], gs[:, 384:512],
                                         AF.Tanh).then_inc(actg_sem, 1)
                    # c update
                    nc.vector.wait_ge(actg_sem, t + 1)
                    if t > 0:
                        nc.vector.wait_ge(acttc_sem, t)
                    nc.vector.tensor_tensor(c_st[:], c_st[:], gs[:, 128:256],
                                            ALU.mult)
                    nc.vector.tensor_tensor(gs[:, 0:128], gs[:, 0:128],
                                            gs[:, 384:512], ALU.mult)
                    nc.vector.tensor_tensor(c_st[:], c_st[:], gs[:, 0:128],
                                            ALU.add).then_inc(dvec_sem, 1)
                    nc.scalar.wait_ge(dvec_sem, t + 1)
                    nc.scalar.activation(tc_t[:], c_st[:],
                                         AF.Tanh).then_inc(acttc_sem, 1)
                    nc.vector.wait_ge(acttc_sem, t + 1)
                    if t > 0:
                        nc.vector.wait_ge(petr_sem, t)
                    nc.vector.tensor_tensor(h_sl[:], gs[:, 256:384], tc_t[:],
                                            ALU.mult).then_inc(dveh_sem, 1)
                    # transpose h slice -> [128, 32]
                    nc.tensor.wait_ge(dveh_sem, t + 1)
                    if t > 1:
                        nc.tensor.wait_ge(actcp_sem, t - 1)
                    nc.tensor.transpose(tp[:], h_sl[:], idf[0:32, 0:32]).then_inc(
                        petr_sem, 1)
                    nc.scalar.wait_ge(petr_sem, t + 1)
                    if t > 1:
                        nc.scalar.wait_ge(send_sem, 16 * (t - 1))
                    nc.scalar.activation(h_stage[:], tp[:],
                                         AF.Copy).then_inc(actcp_sem, 1)
                    # local self-slot write
                    nc.vector.wait_ge(actcp_sem, t + 1)
                    nc.vector.wait_ge(mm_sem, t + 1)
                    nc.vector.tensor_copy(
                        hT_all[:, ds(bufn * 256 + pid_v * 32, 32)],
                        h_stage[:]).then_inc(dveself_sem, 1)
                    # hs history copy (after all remote arrivals of this step)
                    nc.vector.wait_ge(recv_sem, 14 * (t + 1))
                    nc.vector.tensor_copy(
                        hs_sbuf[:, :, t, :],
                        hT_view[:, bufn, :, ds(pid_v * 4, BC)]).then_inc(
                            dvehs_sem, 1)
                    # broadcast to peers
                    nc.gpsimd.wait_ge(actcp_sem, t + 1)
                    if t > 0:
                        nc.gpsimd.wait_ge(dvehs_sem, t)
                    pr = nc.gpsimd.remote_dma_broadcast(
                        hT_all[:, ds(bufn * 256 + pid_gp * 32, 32)],
                        h_stage[:], remote_sem=recv_sem, local_sem=send_sem,
                        rdests=[None] + [(0, k) for k in range(1, 8)])
                    pr.then_inc(prep_sem, 1)
                    nc.gpsimd.wait_ge(prep_sem, t + 1)
                    nc.gpsimd.trigger_dma(count=1)

            # ============== phase 5: logits + nll ==============
            with (
                tc.tile_pool(name="p5", bufs=3) as p5,
                tc.tile_pool(name="p5w", bufs=1) as p5w,
            ):
                oh = p5w.tile([128, 2, CTOK], BF16)
                nc.sync.dma_start(oh[:], onehotT[:])
                tm = p5w.tile([1, CTOK], F32)
                nc.sync.dma_start(tm[:], tmaskT[:])
                wob = p5w.tile([1, V], BF16)
                nc.sync.dma_start(wob[:], wout[DIM + HALF:DIM + HALF + 1, :])
                onesPF = p5w.tile([128, 1], F32)
                nc.vector.memset(onesPF[:], 1.0)
                nllacc = p5w.tile([1, 512], F32)
                nc.vector.memset(nllacc[:], 0.0)
                pid_sy = nc.sync.partition_id()
                wo_t = {}
                for kc in range(12):
                    for vc in range(2):
                        wt5 = p5w.tile([128, 128], BF16, tag=f"wo{kc}_{vc}",
                                       name="wt5")
                        nc.sync.dma_start(
                            wt5[:], wout[kc * 128:(kc + 1) * 128,
                                         vc * 128:(vc + 1) * 128])
                        wo_t[(kc, vc)] = wt5
                for ci in range(8):
                    b_in, half = ci // 2, ci % 2
                    ct0 = half * 500
                    # cs rhs tiles (4 chunks) from csT_all own slice
                    csr = []
                    full = csT_all[ds(pid_sy * 128, 128), 0:20 * TOK]
                    v = full.rearrange("p (s4 d t) -> p s4 d t", s4=SHRINK, d=4)
                    for dc in range(4):
                        rt = p5.tile([128, 100, SHRINK], BF16, tag=f"csr{dc}",
                                     name="rt")
                        for s5 in range(SHRINK):
                            nc.sync.dma_start(
                                rt[:, :, s5],
                                v[:, s5, dc, b_in * T + ct0 // 5:
                                  b_in * T + ct0 // 5 + 100])
                        csr.append(rt)
                    # logits psum per vocab chunk
                    exs = []
                    pks = []
                    for vc in range(2):
                        p = pst("t0" if vc == 0 else "t1")[:, :500]
                        for kc in range(8):
                            nc.tensor.matmul(
                                p, wo_t[(kc, vc)],
                                hs_sbuf[:, kc, ct0:ct0 + 500, b_in],
                                start=(kc == 0), stop=False)
                        for dc in range(4):
                            nc.tensor.matmul(
                                p, wo_t[(8 + dc, vc)],
                                csr[dc][:].rearrange("p t s -> p (t s)"),
                                start=False, stop=False)
                        nc.tensor.matmul(p, wob[:, vc * 128:(vc + 1) * 128],
                                         ones[0:1, :500], start=False, stop=True)
                        ex = p5.tile([128, 500], BF16, tag=f"ex{vc}", name="ex")
                        nc.scalar.activation(ex[:], p, AF.Exp)
                        exs.append(ex)
                        pk = p5.tile([128, 500], BF16, tag=f"pk{vc}", name="pk")
                        nc.vector.tensor_tensor(
                            pk[:], p,
                            oh[:, vc, b_in * S + ct0:b_in * S + ct0 + 500],
                            ALU.mult)
                        pks.append(pk)
                    sps = pst("t4")[:1, :500]
                    for vc in range(2):
                        nc.tensor.matmul(sps, onesP[:], exs[vc][:],
                                         start=(vc == 0), stop=(vc == 1))
                    pps = pst("t5")[:1, :500]
                    for vc in range(2):
                        nc.tensor.matmul(pps, onesP[:], pks[vc][:],
                                         start=(vc == 0), stop=(vc == 1))
                    lse = p5.tile([1, 500], F32, tag="lse", name="lse")
                    nc.scalar.activation(lse[:], sps, AF.Ln)
                    nllc = p5.tile([1, 500], F32, tag="nllc", name="nllc")
                    nc.vector.tensor_tensor(nllc[:], lse[:], pps, ALU.subtract)
                    nc.vector.tensor_tensor(
                        nllc[:], nllc[:],
                        tm[:, b_in * S + ct0:b_in * S + ct0 + 500], ALU.mult)
                    nc.vector.tensor_tensor(nllacc[:, :500], nllacc[:, :500],
                                            nllc[:], ALU.add)
                tot = p5w.tile([1, 1], F32)
                nc.vector.tensor_reduce(tot[:], nllacc[:, :500],
                                        mybir.AxisListType.X, ALU.add)
                nc.sync.dma_start(out_nll[:], tot[:])

    nc.compile()
    return nc
